# revision 11
# baseline (speedup 1.0000x reference)
"""Batched KNN (k=16) + mean feature gather on 8 Trainium2 NeuronCores.

Problem: for each of 16384 query points x (3-D), find the 16 nearest
neighbors among 16384 base points y restricted to the same batch id, and
output the mean of their 16-D features.

Strategy (one core per 2048-query shard; batch-sorted ids give per-batch
locality so each core only needs its own y span — no collectives):

1. Scores S[i,j] = 2*x_i.y_j - |y_j|^2 (row-constant -|x|^2 dropped; order
   preserved) via TensorE matmul in bf16 with 3-term split arithmetic
   (f32-accurate), plus a batch-mismatch penalty -65536*(xb-yb)^2 folded in
   as extra contraction slots (exactly cancels for same-batch pairs).
2. Per-row top-16 threshold on VectorE: max8 per 128-wide group, then
   merge the 8*G group candidates with max8/match_replace to get the 16th
   and 17th largest; threshold t = midpoint.
3. D = S^T - t via a second matmul (j on partitions) with -t as 3 extra
   bf16-split contraction slots (t transposed via a small DRAM roundtrip);
   ScalarE Sign -> w = +/-1 selection matrix in bf16.
4. Gather: out = (w^T @ feats + colsum)/32 = mean of the 16 selected
   features, via TensorE with w as stationary operand.
"""

import os

import numpy as np
import ml_dtypes

import concourse.bass as bass
import concourse.mybir as mybir
from concourse import bacc
from concourse.tile import TileContext
from concourse.bass_utils import run_bass_kernel_spmd

N_CORES = 8
FEAT = 16
PEN = 65536.0
SENTINEL = 16.0  # batch id for padded y rows (real ids are < 8)
NEG_BIG = -3.0e38

bf16 = ml_dtypes.bfloat16

# contraction slot layout
KS = 3 + 3 + 18  # penalty + y^2 splits + 6 product terms per coordinate
T0 = 32          # threshold rows start here (DMA-to-SBUF needs start % 32 == 0)
KD = T0 + 3      # + 3 threshold split slots (S^T - t matmul only)


def _split3(v):
    """3-term bf16 split of a float64 array: v ~ h+m+l, residual ~2^-27 |v|."""
    h = v.astype(bf16)
    r = v - h.astype(np.float64)
    m = r.astype(bf16)
    l = (r - m.astype(np.float64)).astype(bf16)
    return h, m, l


def _build_sides(xc, xbc, yc, ybc):
    """Host prep of the contraction-slot tensors.

    Returns (X [KD, R], Y [KD, C]) bf16. X rows KS..KD-1 are zeros (filled
    on device with the -t splits); Y rows KS..KD-1 are ones.
    """
    R, C = xc.shape[0], yc.shape[0]
    xs, ys = [], []
    xb64 = xbc.astype(np.float64)
    yb64 = ybc.astype(np.float64)
    # batch penalty: accumulates first, exactly cancels when xb == yb
    xs += [-PEN * xb64 * xb64, 2 * PEN * xb64, np.full(R, -PEN)]
    ys += [np.ones(C), yb64, yb64 * yb64]
    # -|y|^2, 3-split
    c = -(yc.astype(np.float64) ** 2).sum(1)
    ch, cm, cl = (t.astype(np.float64) for t in _split3(c))
    xs += [np.ones(R)] * 3
    ys += [ch, cm, cl]
    # products 2*x_k*y_k, 6 split terms per coordinate
    for k in range(3):
        a = 2.0 * xc[:, k].astype(np.float64)
        b = yc[:, k].astype(np.float64)
        ah, am, al = (t.astype(np.float64) for t in _split3(a))
        bh, bm, bl = (t.astype(np.float64) for t in _split3(b))
        for xa, yb_ in [(ah, bh), (ah, bm), (am, bh), (ah, bl), (al, bh), (am, bm)]:
            xs.append(xa)
            ys.append(yb_)
    # zero padding up to T0, then device-filled threshold slots (y side = 1)
    while len(xs) < T0:
        xs.append(np.zeros(R))
        ys.append(np.zeros(C))
    xs += [np.zeros(R)] * 3
    ys += [np.ones(C)] * 3
    X = np.stack([v.astype(bf16) for v in xs])
    Y = np.stack([v.astype(bf16) for v in ys])
    return X, Y


def _build_nc(R, C):
    """Build the Bass graph for one core (SPMD: all cores run this)."""
    rb = R // 128   # query row blocks
    G = C // 128    # candidate groups (max8 group = 128 wide)
    CH = 1024       # candidate sub-tile for the S matmul (2 PSUM banks)
    f32 = mybir.dt.float32
    bft = mybir.dt.bfloat16

    nc = bacc.Bacc(name="knn16")
    xk = nc.dram_tensor("xk", [KD, R], bft, kind="ExternalInput")
    yk = nc.dram_tensor("yk", [KD, C], bft, kind="ExternalInput")
    fe = nc.dram_tensor("fe", [C, FEAT], bft, kind="ExternalInput")
    cs = nc.dram_tensor("cs", [128, rb * FEAT], f32, kind="ExternalInput")
    td = nc.dram_tensor("td", [3 * R], bft, kind="Internal")
    out = nc.dram_tensor("out", [R, FEAT], f32, kind="ExternalOutput")

    with TileContext(nc) as tc:
        with (
            tc.tile_pool(name="const", bufs=1) as const,
            tc.tile_pool(name="spool", bufs=2, space="PSUM") as spool,
            tc.tile_pool(name="dpool", bufs=2, space="PSUM") as dpool,
            tc.tile_pool(name="gpool", bufs=2, space="PSUM") as gpool,
            tc.tile_pool(name="work", bufs=2) as work,
        ):
            xk_sb = const.tile([KD, R], bft)
            yk_sb = const.tile([KD, C], bft)
            fe_sb = const.tile([128, G * FEAT], bft)
            cs_sb = const.tile([128, rb * FEAT], f32)
            tsplit = const.tile([128, 3, rb], bft)

            nc.sync.dma_start(out=xk_sb[0:T0, :], in_=xk[0:T0, :])
            nc.sync.dma_start(out=yk_sb[:, :], in_=yk[:, :])
            nc.sync.dma_start(
                out=fe_sb[:, :].rearrange("p (g f) -> p g f", g=G),
                in_=fe[:, :].rearrange("(g p) f -> p g f", p=128),
            )
            nc.sync.dma_start(out=cs_sb[:, :], in_=cs[:, :])

            # ---- Phase A: scores + per-row top-16 threshold ----
            for b in range(rb):
                cand = work.tile([128, G * 8], f32, tag="cand")
                for h in range(C // CH):
                    s_ps = spool.tile([128, CH], f32, tag="S")
                    for q in range(CH // 512):
                        nc.tensor.matmul(
                            s_ps[:, q * 512:(q + 1) * 512],
                            lhsT=xk_sb[0:KS, b * 128:(b + 1) * 128],
                            rhs=yk_sb[0:KS, h * CH + q * 512:h * CH + (q + 1) * 512],
                            start=True,
                            stop=True,
                        )
                    for g in range(CH // 128):
                        gi = h * (CH // 128) + g
                        nc.vector.max(
                            out=cand[:, gi * 8:(gi + 1) * 8],
                            in_=s_ps[:, g * 128:(g + 1) * 128],
                        )
                # merge: top-17 of the 8*G group winners
                m1 = work.tile([128, 8], f32, tag="m1")
                nc.vector.max(out=m1, in_=cand)
                cand2 = work.tile([128, G * 8], f32, tag="cand2")
                nc.vector.match_replace(
                    out=cand2, in_to_replace=m1, in_values=cand, imm_value=NEG_BIG
                )
                m2 = work.tile([128, 8], f32, tag="m2")
                nc.vector.max(out=m2, in_=cand2)
                cand3 = work.tile([128, G * 8], f32, tag="cand3")
                nc.vector.match_replace(
                    out=cand3, in_to_replace=m2, in_values=cand2, imm_value=NEG_BIG
                )
                v17 = work.tile([128, 1], f32, tag="v17")
                nc.vector.tensor_reduce(
                    out=v17, in_=cand3, axis=mybir.AxisListType.X,
                    op=mybir.AluOpType.max,
                )
                # tneg = -(v16 + v17)/2, then 3-term bf16 split into tsplit
                tneg = work.tile([128, 1], f32, tag="tneg")
                nc.vector.tensor_add(out=tneg, in0=m2[:, 7:8], in1=v17)
                nc.vector.tensor_scalar_mul(tneg, tneg, -0.5)
                r32 = work.tile([128, 1], f32, tag="r32")
                nc.vector.tensor_copy(out=tsplit[:, 0, b:b + 1], in_=tneg)
                nc.vector.tensor_copy(out=r32, in_=tsplit[:, 0, b:b + 1])
                nc.vector.tensor_sub(out=tneg, in0=tneg, in1=r32)
                nc.vector.tensor_copy(out=tsplit[:, 1, b:b + 1], in_=tneg)
                nc.vector.tensor_copy(out=r32, in_=tsplit[:, 1, b:b + 1])
                nc.vector.tensor_sub(out=tneg, in0=tneg, in1=r32)
                nc.vector.tensor_copy(out=tsplit[:, 2, b:b + 1], in_=tneg)

            # ---- Phase B: transpose t via DRAM roundtrip ----
            # td flat layout: addr = s*R + b*128 + p  (s = split index)
            with nc.allow_non_contiguous_dma("t transpose scatter"):
                nc.sync.dma_start(
                    out=bass.AP(td, 0, [[1, 128], [R, 3], [128, rb]]),
                    in_=tsplit[:, :, :],
                )
            nc.sync.dma_start(
                out=xk_sb[T0:KD, :],
                in_=bass.AP(td, 0, [[R, 3], [1, R]]),
            )

            # ---- Phase C1: D = S^T - t, sign -> w_all (+/-1, bf16) ----
            w_all = const.tile([128, G, R], bft)
            for jc in range(G):
                for h in range(R // 512):
                    d_ps = dpool.tile([128, 512], f32, tag="D")
                    nc.tensor.matmul(
                        d_ps,
                        lhsT=yk_sb[0:KD, jc * 128:(jc + 1) * 128],
                        rhs=xk_sb[0:KD, h * 512:(h + 1) * 512],
                        start=True,
                        stop=True,
                    )
                    nc.scalar.activation(
                        out=w_all[:, jc, h * 512:(h + 1) * 512],
                        in_=d_ps,
                        func=mybir.ActivationFunctionType.Sign,
                    )

            # ---- Phase C2: gather, one i-block per PSUM bank group ----
            out_sb = const.tile([128, rb * FEAT], f32)
            for ib in range(rb):
                g_ps = gpool.tile([128, FEAT], f32, tag="G")
                for jc in range(G):
                    nc.tensor.matmul(
                        g_ps,
                        lhsT=w_all[:, jc, ib * 128:(ib + 1) * 128],
                        rhs=fe_sb[:, jc * FEAT:(jc + 1) * FEAT],
                        start=(jc == 0),
                        stop=(jc == G - 1),
                    )
                nc.vector.scalar_tensor_tensor(
                    out=out_sb[:, ib * FEAT:(ib + 1) * FEAT],
                    in0=g_ps,
                    scalar=1.0 / 32.0,
                    in1=cs_sb[:, ib * FEAT:(ib + 1) * FEAT],
                    op0=mybir.AluOpType.mult,
                    op1=mybir.AluOpType.add,
                )
            nc.sync.dma_start(
                out=out[:, :].rearrange("(b p) f -> p b f", p=128),
                in_=out_sb[:, :].rearrange("p (b f) -> p b f", b=rb),
            )
    nc.finalize()
    return nc


_NC_CACHE = {}


def _get_nc(R, C):
    key = (R, C)
    if key not in _NC_CACHE:
        _NC_CACHE[key] = _build_nc(R, C)
    return _NC_CACHE[key]


def kernel(x, y, y_atomflex, x_batch, y_batch):
    x = np.ascontiguousarray(np.asarray(x, dtype=np.float32))
    y = np.ascontiguousarray(np.asarray(y, dtype=np.float32))
    feats = np.ascontiguousarray(np.asarray(y_atomflex, dtype=np.float32))
    in_dtype = np.asarray(x_batch).dtype
    xb = np.asarray(x_batch).astype(np.int64)
    yb = np.asarray(y_batch).astype(np.int64)

    N = x.shape[0]
    R = N // N_CORES

    # per-core y spans (batch ids are sorted)
    spans = []
    for c in range(N_CORES):
        blo, bhi = xb[c * R], xb[(c + 1) * R - 1]
        s = int(np.searchsorted(yb, blo, "left"))
        e = int(np.searchsorted(yb, bhi, "right"))
        spans.append((s, e))
    C = max(1024, -(-max(e - s for s, e in spans) // 1024) * 1024)

    in_maps = []
    for c in range(N_CORES):
        s, e = spans[c]
        n = e - s
        yc = np.zeros((C, 3), np.float32)
        yc[:n] = y[s:e]
        ybc = np.full(C, SENTINEL)
        ybc[:n] = yb[s:e]
        fec = np.zeros((C, FEAT), np.float32)
        fec[:n] = feats[s:e]
        fe_bf = fec.astype(bf16)
        X, Y = _build_sides(x[c * R:(c + 1) * R], xb[c * R:(c + 1) * R], yc, ybc)
        colsum = (fe_bf.astype(np.float64).sum(0) / 32.0).astype(np.float32)
        cs = np.ascontiguousarray(np.tile(colsum[None, :], (128, R // 128)))
        in_maps.append(
            {
                "xk": np.ascontiguousarray(X),
                "yk": np.ascontiguousarray(Y),
                "fe": np.ascontiguousarray(fe_bf),
                "cs": cs,
            }
        )

    nc = _get_nc(R, C)
    trace = bool(int(os.environ.get("KNN_TRACE", "0")))
    res = run_bass_kernel_spmd(
        nc, in_maps, core_ids=list(range(N_CORES)), trace=trace
    )
    if trace and res.exec_time_ns is not None:
        print(f"HW exec time: {res.exec_time_ns} ns")
        if res.instructions_and_trace is not None:
            print(f"trace: {res.instructions_and_trace[1]}")

    out = np.concatenate([r["out"] for r in res.results], axis=0)
    return out.astype(np.float32)


if __name__ == "__main__":
    # smoke test against a tiny local reference
    import reference

    inputs = {k: np.asarray(v) for k, v in reference.setup_inputs().items()}
    expected = np.asarray(reference.reference(**inputs))
    actual = kernel(**inputs)
    err = np.linalg.norm(actual - expected) / np.linalg.norm(expected)
    print(f"Relative error: {err:.6f}")
